# revision 7
# baseline (speedup 1.0000x reference)
"""Causal self-attention (RoPE) Trainium2 kernel, 8-way sharded.

Sharding: core = (batch b in 0..1) x (head group g in 0..3, 4 heads each).
Each core computes its batch's attention for its 4 heads plus the partial
output projection; the host sums the 4 partials per batch.

Layout strategy (per core):
- host passes xT = x[b].T so the embed dim lands on SBUF partitions.
- W_qkv columns are permuted so q^T/k^T emerge from the projection matmul
  already transposed, with RoPE even/odd dim pairs de-interleaved into
  x1/x2 partition blocks (scores are invariant to a head-dim permutation).
- scores are computed transposed (sT[j,i]); softmax needs no max pass
  (|scores| < ~4) and the denominator is obtained by appending a ones
  column to V (M=65 PV matmuls). Normalization is applied to the context
  after PV via a K=1 broadcast matmul and Ln/Exp reciprocal.
- causal masking: only j<=i column ranges are computed; the single
  diagonal 128x128 block per (j-tile, i-block) gets a triangle multiply.
"""
import sys

sys.path.insert(0, "/opt/trn_rl_repo")

import numpy as np

NUM_HEADS = 16
HEAD_DIM = 64
B, S, E = 2, 2048, 1024
HG = 4                      # heads per core
NG = NUM_HEADS // HG        # head groups
N_CORES = B * NG
F_QK = 2 * HG * HEAD_DIM    # 512 projected q+k rows per core
F_V = HG * HEAD_DIM         # 256 v cols per core
ESUB = E // 128             # 8 K-subtiles over embed dim
NCHUNK = 4                  # 512-col seq chunks
CHUNK = S // NCHUNK         # 512
NST = S // 128              # 16 seq tiles of 128

_CACHE = {}


def _build_program():
    import concourse.bass as bass
    import concourse.mybir as mybir
    import concourse.tile as tile
    from concourse import bacc

    f32 = mybir.dt.float32
    Alu = mybir.AluOpType
    Act = mybir.ActivationFunctionType

    nc = bacc.Bacc("TRN2", target_bir_lowering=False, debug=False,
                   num_devices=N_CORES)

    xT_d = nc.dram_tensor("xT", (E, S), f32, kind="ExternalInput").ap()
    wqk_d = nc.dram_tensor("wqk", (E, F_QK), f32, kind="ExternalInput").ap()
    wv_d = nc.dram_tensor("wv", (E, F_V), f32, kind="ExternalInput").ap()
    wout_d = nc.dram_tensor("wout", (F_V, E), f32, kind="ExternalInput").ap()
    cs_d = nc.dram_tensor("cs", (128, S), f32, kind="ExternalInput").ap()
    sn_d = nc.dram_tensor("sn", (128, S), f32, kind="ExternalInput").ap()
    tri_d = nc.dram_tensor("tri", (128, 128), f32, kind="ExternalInput").ap()
    out_d = nc.dram_tensor("out", (S, E), f32, kind="ExternalOutput").ap()

    with tile.TileContext(nc) as tc:
        with tc.tile_pool(name="wc", bufs=1) as wpool, \
             tc.tile_pool(name="vctx", bufs=1) as vpool:
            # ---- small whole-kernel constants ----
            tri_sb = wpool.tile([128, 128], f32)
            nc.sync.dma_start(tri_sb[:], tri_d[:])
            ones64 = wpool.tile([1, 64], f32)
            nc.vector.memset(ones64[:], 1.0)

            # v tiles with per-head ones column: [128, st, 4*65]
            v_sb = vpool.tile([128, NST, HG * 65], f32)
            nc.gpsimd.memset(v_sb[:], 1.0)

            # rope outputs (x1/x2 aligned); freed after relayout
            rotcm = tc.tile_pool(name="rot", bufs=1)
            rotpool = rotcm.__enter__()
            qra = rotpool.tile([128, S], f32)
            qrb = rotpool.tile([128, S], f32)
            kra = rotpool.tile([128, S], f32)
            krb = rotpool.tile([128, S], f32)

            # ---- phase 1: projections ----
            with tc.tile_pool(name="x", bufs=1) as xpool, \
                 tc.tile_pool(name="rsc", bufs=3) as rsc:
                wqk_sb = xpool.tile([128, ESUB, F_QK], f32)
                nc.sync.dma_start(wqk_sb[:], wqk_d.rearrange("(o p) f -> p o f", p=128))
                wv_sb = xpool.tile([128, ESUB, F_V], f32)
                nc.sync.dma_start(wv_sb[:], wv_d.rearrange("(o p) f -> p o f", p=128))
                cs_sb = xpool.tile([128, S], f32)
                nc.sync.dma_start(cs_sb[:], cs_d[:])
                sn_sb = xpool.tile([128, S], f32)
                nc.sync.dma_start(sn_sb[:], sn_d[:])
                xT_sb = xpool.tile([128, ESUB, S], f32)
                for c in range(NCHUNK):
                    csl = slice(c * CHUNK, (c + 1) * CHUNK)
                    nc.sync.dma_start(
                        xT_sb[:, :, csl],
                        xT_d.rearrange("(o p) s -> p o s", p=128)[:, :, csl])

                # q/k projection + rope, chunked over seq
                with tc.tile_pool(name="ps_qk", bufs=2, space="PSUM") as psqk:
                    for c in range(NCHUNK):
                        csl = slice(c * CHUNK, (c + 1) * CHUNK)
                        pqa = psqk.tile([128, CHUNK], f32, tag="qa")
                        pqb = psqk.tile([128, CHUNK], f32, tag="qb")
                        pka = psqk.tile([128, CHUNK], f32, tag="ka")
                        pkb = psqk.tile([128, CHUNK], f32, tag="kb")
                        for e in range(ESUB):
                            kw = dict(start=(e == 0), stop=(e == ESUB - 1))
                            xs = xT_sb[:, e, csl]
                            nc.tensor.matmul(pqa[:], wqk_sb[:, e, 0:128], xs, **kw)
                            nc.tensor.matmul(pqb[:], wqk_sb[:, e, 128:256], xs, **kw)
                            nc.tensor.matmul(pka[:], wqk_sb[:, e, 256:384], xs, **kw)
                            nc.tensor.matmul(pkb[:], wqk_sb[:, e, 384:512], xs, **kw)
                        for (pa, pb, ra, rb) in ((pqa, pqb, qra, qrb),
                                                 (pka, pkb, kra, krb)):
                            t1 = rsc.tile([128, CHUNK], f32, tag="t1", name="t1")
                            t2 = rsc.tile([128, CHUNK], f32, tag="t2", name="t2")
                            nc.vector.tensor_tensor(t1[:], pa[:], cs_sb[:, csl], Alu.mult)
                            nc.vector.tensor_tensor(t2[:], pb[:], sn_sb[:, csl], Alu.mult)
                            nc.vector.tensor_tensor(ra[:, csl], t1[:], t2[:], Alu.subtract)
                            t3 = rsc.tile([128, CHUNK], f32, tag="t1", name="t3")
                            t4 = rsc.tile([128, CHUNK], f32, tag="t2", name="t4")
                            nc.vector.tensor_tensor(t3[:], pa[:], sn_sb[:, csl], Alu.mult)
                            nc.vector.tensor_tensor(t4[:], pb[:], cs_sb[:, csl], Alu.mult)
                            nc.vector.tensor_tensor(rb[:, csl], t3[:], t4[:], Alu.add)

                # v projection per seq tile: v[s,f] = xT.T @ Wv
                with tc.tile_pool(name="ps_v", bufs=2, space="PSUM") as psv:
                    for st in range(NST):
                        ssl = slice(st * 128, (st + 1) * 128)
                        pv = psv.tile([128, F_V], f32, tag="v")
                        for e in range(ESUB):
                            nc.tensor.matmul(pv[:], xT_sb[:, e, ssl], wv_sb[:, e, :],
                                             start=(e == 0), stop=(e == ESUB - 1))
                        # scatter heads into 65-wide slots (col 64 etc stay 1.0)
                        nc.vector.tensor_copy(
                            v_sb[:, st, :].rearrange("p (h w) -> p h w", h=HG)[:, :, 0:64],
                            pv[:].rearrange("p (h w) -> p h w", h=HG))

            # ---- phase 1.5: pair-interleave relayout (SBUF->SBUF DMA) ----
            # pair tile rows: [h_even x1 | h_even x2 | h_odd x1 | h_odd x2]
            paircm = tc.tile_pool(name="pairs", bufs=1)
            pairpool = paircm.__enter__()
            qp = pairpool.tile([128, 2, S], f32)   # pair-interleaved q^T
            kp = pairpool.tile([128, 2, S], f32)
            ctxn_sb = pairpool.tile([128, 2, S], f32)
            wout_sb = pairpool.tile([128, 2, E], f32)
            nc.sync.dma_start(wout_sb[:], wout_d.rearrange("(o p) e -> p o e", p=128))
            for p in range(2):
                h0, h1 = 2 * p, 2 * p + 1
                for (ra, rb, dst) in ((qra, qrb, qp), (kra, krb, kp)):
                    nc.sync.dma_start(dst[0:32, p, :], ra[32 * h0:32 * h0 + 32, :])
                    nc.sync.dma_start(dst[32:64, p, :], rb[32 * h0:32 * h0 + 32, :])
                    nc.sync.dma_start(dst[64:96, p, :], ra[32 * h1:32 * h1 + 32, :])
                    nc.sync.dma_start(dst[96:128, p, :], rb[32 * h1:32 * h1 + 32, :])

            # ---- phase 2: attention ----
            scale = 1.0 / float(np.sqrt(HEAD_DIM))
            with tc.tile_pool(name="ps_at", bufs=1, space="PSUM") as psat, \
                 tc.tile_pool(name="ps_z", bufs=1, space="PSUM") as psz, \
                 tc.tile_pool(name="pt", bufs=3) as ptp, \
                 tc.tile_pool(name="nrm", bufs=2) as nrm:
                for p in range(2):
                    for b in range(NCHUNK):
                        i0 = b * CHUNK
                        njt = 4 * b + 4
                        ctx = [psat.tile([65, CHUNK], f32, tag=f"ctx{a}",
                                         name=f"ctx{a}")
                               for a in range(2)]
                        for jt in range(njt):
                            r = jt - 4 * b
                            off = 128 * max(r, 0)
                            w = CHUNK - off
                            isl = slice(i0 + off, i0 + CHUNK)
                            for a in range(2):
                                ps_s = psat.tile([128, CHUNK], f32,
                                                 tag=f"s{a}", bufs=2, name="ps_s")
                                nc.tensor.matmul(
                                    ps_s[:, :w],
                                    kp[64 * a:64 * a + 64, p,
                                       128 * jt:128 * jt + 128],
                                    qp[64 * a:64 * a + 64, p, isl],
                                    start=True, stop=True)
                                pt = ptp.tile([128, CHUNK], f32, tag=f"p{a}",
                                              name="pt")
                                nc.scalar.activation(pt[:, :w], ps_s[:, :w],
                                                     Act.Exp, scale=scale)
                                if r >= 0:
                                    nc.vector.tensor_tensor(
                                        pt[:, 0:128], pt[:, 0:128], tri_sb[:],
                                        Alu.mult)
                                nc.tensor.matmul(
                                    ctx[a][:, off:], v_sb[:, jt, 65 * (2 * p + a):
                                                          65 * (2 * p + a) + 65],
                                    pt[:, :w],
                                    start=(jt == 0), stop=(jt == njt - 1))
                        # normalize: ctx[0:64] * exp(-ln(Z)), Z = ctx row 64
                        for a in range(2):
                            zrow = nrm.tile([1, CHUNK], f32, tag="zrow", name="zrow")
                            nc.vector.tensor_copy(zrow[:], ctx[a][64:65, :])
                            zb = psz.tile([64, CHUNK], f32, tag="zb", name="zb")
                            nc.tensor.matmul(zb[:], ones64[:], zrow[:],
                                             start=True, stop=True)
                            lnz = nrm.tile([64, CHUNK], f32, tag="lnz", name="lnz")
                            nc.scalar.activation(lnz[:], zb[:], Act.Ln)
                            rz = nrm.tile([64, CHUNK], f32, tag="rz", name="rz")
                            nc.scalar.activation(rz[:], lnz[:], Act.Exp, scale=-1.0)
                            nc.vector.tensor_tensor(
                                ctxn_sb[64 * a:64 * a + 64, p, i0:i0 + CHUNK],
                                ctx[a][0:64, :], rz[:], Alu.mult)

            # ---- phase 3: output projection ----
            with tc.tile_pool(name="ps_o", bufs=4, space="PSUM") as pso, \
                 tc.tile_pool(name="ot", bufs=4) as otp:
                for st in range(NST):
                    ssl = slice(st * 128, (st + 1) * 128)
                    for n in range(2):
                        nsl = slice(n * 512, (n + 1) * 512)
                        po = pso.tile([128, 512], f32, tag="po", name="po")
                        nc.tensor.matmul(po[:], ctxn_sb[:, 0, ssl],
                                         wout_sb[:, 0, nsl], start=True, stop=False)
                        nc.tensor.matmul(po[:], ctxn_sb[:, 1, ssl],
                                         wout_sb[:, 1, nsl], start=False, stop=True)
                        ot = otp.tile([128, 512], f32, tag="ot", name="ot")
                        nc.any.tensor_copy(ot[:], po[:])
                        nc.sync.dma_start(out_d[ssl, nsl], ot[:])
            paircm.__exit__(None, None, None)
            rotcm.__exit__(None, None, None)

    nc.compile()
    return nc


def _host_inputs(x, W_qkv, W_out):
    """Build the 8 per-core input maps."""
    x = np.ascontiguousarray(np.asarray(x, dtype=np.float32))
    W_qkv = np.asarray(W_qkv, dtype=np.float32)
    W_out = np.asarray(W_out, dtype=np.float32)

    pos = np.arange(S)
    freqs = 1.0 / 10000.0 ** (np.arange(0, HEAD_DIM, 2) / HEAD_DIM)
    ang = pos[:, None] * freqs[None, :]            # (S, 32)
    cs32 = np.cos(ang).T.astype(np.float32)        # (32, S)
    sn32 = np.sin(ang).T.astype(np.float32)
    cs = np.tile(cs32, (4, 1))                     # (128, S)
    sn = np.tile(sn32, (4, 1))
    tri = (np.arange(128)[:, None] <= np.arange(128)[None, :]).astype(np.float32)

    in_maps = []
    for b in range(B):
        xT = np.ascontiguousarray(x[b].T)
        for g in range(NG):
            heads = np.arange(HG * g, HG * g + HG)
            qa = np.concatenate([0 * NUM_HEADS * HEAD_DIM + h * HEAD_DIM
                                 + np.arange(0, HEAD_DIM, 2) for h in heads])
            qb = qa + 1
            ka = qa + NUM_HEADS * HEAD_DIM
            kb = ka + 1
            wqk = np.ascontiguousarray(
                W_qkv[:, np.concatenate([qa, qb, ka, kb])])
            vcols = np.concatenate([2 * NUM_HEADS * HEAD_DIM + h * HEAD_DIM
                                    + np.arange(HEAD_DIM) for h in heads])
            wv = np.ascontiguousarray(W_qkv[:, vcols])
            wout = np.ascontiguousarray(
                W_out[HG * g * HEAD_DIM:HG * (g + 1) * HEAD_DIM])
            in_maps.append({"xT": xT, "wqk": wqk, "wv": wv, "wout": wout,
                            "cs": cs, "sn": sn, "tri": tri})
    return in_maps


def get_program():
    if "nc" not in _CACHE:
        _CACHE["nc"] = _build_program()
    return _CACHE["nc"]


def run(x, W_qkv, W_out, trace=False, tmpdir=None):
    from concourse import bass_utils
    nc = get_program()
    in_maps = _host_inputs(x, W_qkv, W_out)
    res = bass_utils.run_bass_kernel_spmd(
        nc, in_maps, core_ids=list(range(N_CORES)), trace=trace, tmpdir=tmpdir)
    out = np.zeros((B, S, E), np.float32)
    for b in range(B):
        for g in range(NG):
            out[b] += res.results[b * NG + g]["out"]
    return out, res


def kernel(x, W_qkv, W_out):
    out, _ = run(x, W_qkv, W_out)
    return out


# revision 13
# speedup vs baseline: 1.9378x; 1.9378x over previous
"""Causal self-attention (RoPE) Trainium2 kernel, 8-way sharded.

Sharding: core = (batch b in 0..1) x (head group g in 0..3, 4 heads each).
Each core computes its batch's attention for its 4 heads plus the partial
output projection; the host sums the 4 partials per batch.

Layout strategy (per core):
- host passes xT = x[b].T (fp16) so the embed dim lands on SBUF partitions.
- W_qkv columns are permuted so q^T/k^T emerge from the projection matmul
  already transposed, with RoPE even/odd dim pairs de-interleaved into
  x1/x2 partition blocks (scores are invariant to a head-dim permutation).
- all matmul operands are fp16 (1 cycle/row on PE vs 4 for fp32); PSUM
  accumulation stays fp32. End-to-end error ~5e-4.
- scores are computed transposed (sT[j,i]); softmax needs no max pass
  (|scores| < ~4) and the denominator is obtained by appending a ones
  column to V (M=65 PV matmuls). Normalization happens at the end via a
  selector-matmul broadcast of Z and Ln/Exp reciprocal on ACT.
- causal masking: only j<=i column ranges are computed; the single
  diagonal 128x128 block per j-tile gets a triangle multiply.
"""
import sys

sys.path.insert(0, "/opt/trn_rl_repo")

import numpy as np

NUM_HEADS = 16
HEAD_DIM = 64
B, S, E = 2, 2048, 1024
HG = 4                      # heads per core
NG = NUM_HEADS // HG        # head groups
N_CORES = B * NG
F_QK = 2 * HG * HEAD_DIM    # 512 projected q+k rows per core
F_V = HG * HEAD_DIM         # 256 v cols per core
ESUB = E // 128             # 8 K-subtiles over embed dim
NCHUNK = 4                  # 512-col seq chunks (projection)
CHUNK = S // NCHUNK         # 512
NST = S // 128              # 16 seq tiles of 128
BLK = 1024                  # attention i-block width
NBLK = S // BLK             # 2

_CACHE = {}


def _build_program():
    import concourse.bass as bass
    import concourse.mybir as mybir
    import concourse.tile as tile
    from concourse import bacc

    f32 = mybir.dt.float32
    f16 = mybir.dt.float16
    Alu = mybir.AluOpType
    Act = mybir.ActivationFunctionType

    nc = bacc.Bacc("TRN2", target_bir_lowering=False, debug=False,
                   num_devices=N_CORES)

    xT_d = nc.dram_tensor("xT", (E, S), f16, kind="ExternalInput").ap()
    wqk_d = nc.dram_tensor("wqk", (E, F_QK), f16, kind="ExternalInput").ap()
    wv_d = nc.dram_tensor("wv", (E, F_V), f16, kind="ExternalInput").ap()
    wout_d = nc.dram_tensor("wout", (F_V, E), f16, kind="ExternalInput").ap()
    cs_d = nc.dram_tensor("cs", (128, S), f32, kind="ExternalInput").ap()
    sn_d = nc.dram_tensor("sn", (128, S), f32, kind="ExternalInput").ap()
    tri_d = nc.dram_tensor("tri", (128, 128), f16, kind="ExternalInput").ap()
    sel_d = nc.dram_tensor("sel", (4, 256), f16, kind="ExternalInput").ap()
    out_d = nc.dram_tensor("out", (S, E), f32, kind="ExternalOutput").ap()

    with tile.TileContext(nc) as tc:
        with tc.tile_pool(name="wc", bufs=1) as wpool, \
             tc.tile_pool(name="vctx", bufs=1) as vpool:
            # ---- small whole-kernel constants ----
            tri_sb = wpool.tile([128, 128], f16)
            nc.sync.dma_start(tri_sb[:], tri_d[:])
            sel_sb = wpool.tile([4, 256], f16)
            nc.sync.dma_start(sel_sb[:], sel_d[:])

            # v tiles with per-head ones column: [128, st, 4*65]
            v_sb = vpool.tile([128, NST, HG * 65], f16)
            nc.gpsimd.memset(v_sb[:], 1.0)
            # unnormalized ctx^T (fp16) + Z rows
            ctxu_sb = vpool.tile([128, 2, S], f16)
            zall_sb = vpool.tile([4, S], f32)

            # rope outputs (x1/x2 aligned); pair-interleaved after relayout
            rotcm = tc.tile_pool(name="rot", bufs=1)
            rotpool = rotcm.__enter__()
            qra = rotpool.tile([128, S], f16)
            qrb = rotpool.tile([128, S], f16)
            kra = rotpool.tile([128, S], f16)
            krb = rotpool.tile([128, S], f16)

            # ---- phase 1: projections ----
            with tc.tile_pool(name="x", bufs=1) as xpool, \
                 tc.tile_pool(name="rsc", bufs=3) as rsc:
                wqk_sb = xpool.tile([128, ESUB, F_QK], f16)
                nc.sync.dma_start(wqk_sb[:], wqk_d.rearrange("(o p) f -> p o f", p=128))
                wv_sb = xpool.tile([128, ESUB, F_V], f16)
                nc.sync.dma_start(wv_sb[:], wv_d.rearrange("(o p) f -> p o f", p=128))
                cs_sb = xpool.tile([128, S], f32)
                nc.sync.dma_start(cs_sb[:], cs_d[:])
                sn_sb = xpool.tile([128, S], f32)
                nc.sync.dma_start(sn_sb[:], sn_d[:])
                xT_sb = xpool.tile([128, ESUB, S], f16)
                for c in range(NCHUNK):
                    csl = slice(c * CHUNK, (c + 1) * CHUNK)
                    nc.sync.dma_start(
                        xT_sb[:, :, csl],
                        xT_d.rearrange("(o p) s -> p o s", p=128)[:, :, csl])

                # q/k projection + rope, chunked over seq
                with tc.tile_pool(name="ps_qk", bufs=2, space="PSUM") as psqk:
                    for c in range(NCHUNK):
                        csl = slice(c * CHUNK, (c + 1) * CHUNK)
                        pqa = psqk.tile([128, CHUNK], f32, tag="qa")
                        pqb = psqk.tile([128, CHUNK], f32, tag="qb")
                        pka = psqk.tile([128, CHUNK], f32, tag="ka")
                        pkb = psqk.tile([128, CHUNK], f32, tag="kb")
                        for e in range(ESUB):
                            kw = dict(start=(e == 0), stop=(e == ESUB - 1))
                            xs = xT_sb[:, e, csl]
                            nc.tensor.matmul(pqa[:], wqk_sb[:, e, 0:128], xs, **kw)
                            nc.tensor.matmul(pqb[:], wqk_sb[:, e, 128:256], xs, **kw)
                            nc.tensor.matmul(pka[:], wqk_sb[:, e, 256:384], xs, **kw)
                            nc.tensor.matmul(pkb[:], wqk_sb[:, e, 384:512], xs, **kw)
                        for (pa, pb, ra, rb) in ((pqa, pqb, qra, qrb),
                                                 (pka, pkb, kra, krb)):
                            t1 = rsc.tile([128, CHUNK], f32, tag="t1", name="t1")
                            t2 = rsc.tile([128, CHUNK], f32, tag="t2", name="t2")
                            nc.vector.tensor_tensor(t1[:], pa[:], cs_sb[:, csl], Alu.mult)
                            nc.vector.tensor_tensor(t2[:], pb[:], sn_sb[:, csl], Alu.mult)
                            nc.vector.tensor_tensor(ra[:, csl], t1[:], t2[:], Alu.subtract)
                            t3 = rsc.tile([128, CHUNK], f32, tag="t1", name="t3")
                            t4 = rsc.tile([128, CHUNK], f32, tag="t2", name="t4")
                            nc.vector.tensor_tensor(t3[:], pa[:], sn_sb[:, csl], Alu.mult)
                            nc.vector.tensor_tensor(t4[:], pb[:], cs_sb[:, csl], Alu.mult)
                            nc.vector.tensor_tensor(rb[:, csl], t3[:], t4[:], Alu.add)

                # v projection per seq tile: v[s,f] = xT.T @ Wv
                with tc.tile_pool(name="ps_v", bufs=2, space="PSUM") as psv:
                    for st in range(NST):
                        ssl = slice(st * 128, (st + 1) * 128)
                        pv = psv.tile([128, F_V], f32, tag="v")
                        for e in range(ESUB):
                            nc.tensor.matmul(pv[:], xT_sb[:, e, ssl], wv_sb[:, e, :],
                                             start=(e == 0), stop=(e == ESUB - 1))
                        # scatter heads into 65-wide slots (col 64 etc stay 1.0)
                        nc.vector.tensor_copy(
                            v_sb[:, st, :].rearrange("p (h w) -> p h w", h=HG)[:, :, 0:64],
                            pv[:].rearrange("p (h w) -> p h w", h=HG))

            # ---- phase 1.5: pair-interleave relayout (SBUF->SBUF DMA) ----
            # pair tile rows: [h_even x1 | h_even x2 | h_odd x1 | h_odd x2]
            paircm = tc.tile_pool(name="pairs", bufs=1)
            pairpool = paircm.__enter__()
            qp = pairpool.tile([128, 2, S], f16)   # pair-interleaved q^T
            kp = pairpool.tile([128, 2, S], f16)
            wout_sb = pairpool.tile([128, 2, E], f16)
            nc.sync.dma_start(wout_sb[:], wout_d.rearrange("(o p) e -> p o e", p=128))
            for p in range(2):
                h0, h1 = 2 * p, 2 * p + 1
                for (ra, rb, dst) in ((qra, qrb, qp), (kra, krb, kp)):
                    nc.sync.dma_start(dst[0:32, p, :], ra[32 * h0:32 * h0 + 32, :])
                    nc.sync.dma_start(dst[32:64, p, :], rb[32 * h0:32 * h0 + 32, :])
                    nc.sync.dma_start(dst[64:96, p, :], ra[32 * h1:32 * h1 + 32, :])
                    nc.sync.dma_start(dst[96:128, p, :], rb[32 * h1:32 * h1 + 32, :])

            # ---- phase 2: attention ----
            scale = 1.0 / float(np.sqrt(HEAD_DIM))
            with tc.tile_pool(name="ps_at", bufs=1, space="PSUM") as psat, \
                 tc.tile_pool(name="pt", bufs=2) as ptp:
                for bb in range(NBLK):
                    i0 = bb * BLK
                    njt = 8 * bb + 8
                    for p in range(2):
                        ctx = [psat.tile([65, BLK], f32, tag=f"ctx{a}",
                                         name=f"ctx{a}")
                               for a in range(2)]
                        for jt in range(njt):
                            r = jt - 8 * bb
                            off = 128 * max(r, 0)
                            w = BLK - off
                            # both heads' scores in one [128, 2, BLK] psum
                            ps_s = psat.tile([128, 2, BLK], f32, tag="s",
                                             name="ps_s")
                            chunks = []
                            c0 = off
                            while c0 < BLK:
                                c1 = min(BLK, (c0 // 512 + 1) * 512)
                                chunks.append((c0, c1 - c0))
                                c0 = c1
                            for a in range(2):
                                for ch, cw in chunks:
                                    nc.tensor.matmul(
                                        ps_s[:, a, ch:ch + cw],
                                        kp[64 * a:64 * a + 64, p,
                                           128 * jt:128 * jt + 128],
                                        qp[64 * a:64 * a + 64, p,
                                           i0 + ch:i0 + ch + cw],
                                        start=True, stop=True)
                            pt = ptp.tile([128, 2, BLK], f16, tag="pt", name="pt")
                            nc.scalar.activation(pt[:, :, off:], ps_s[:, :, off:],
                                                 Act.Exp, scale=scale)
                            if r >= 0:
                                nc.vector.tensor_tensor(
                                    pt[:, :, off:off + 128],
                                    pt[:, :, off:off + 128],
                                    tri_sb[:, None, :].to_broadcast((128, 2, 128)),
                                    Alu.mult)
                            # per-element has_written handles the ragged
                            # causal column ranges; the 2KB-granularity group
                            # check cannot express them, so skip it
                            for a in range(2):
                                for ch, cw in chunks:
                                    nc.tensor.matmul(
                                        ctx[a][:, ch:ch + cw],
                                        v_sb[:, jt, 65 * (2 * p + a):
                                             65 * (2 * p + a) + 65],
                                        pt[:, a, ch:ch + cw],
                                        start=(jt == 0), stop=(jt == njt - 1),
                                        skip_group_check=True)
                        # stash unnormalized ctx + Z rows
                        for a in range(2):
                            nc.vector.tensor_copy(
                                ctxu_sb[64 * a:64 * a + 64, p, i0:i0 + BLK],
                                ctx[a][0:64, :])
                            zrow = ptp.tile([1, BLK], f32, tag="zrow",
                                            name="zrow")
                            nc.vector.tensor_copy(zrow[:], ctx[a][64:65, :])
                            nc.sync.dma_start(
                                zall_sb[2 * p + a:2 * p + a + 1, i0:i0 + BLK],
                                zrow[:])

            # ---- phase 2.5: batched normalization ----
            with tc.tile_pool(name="ps_z", bufs=2, space="PSUM") as psz, \
                 tc.tile_pool(name="nrm", bufs=2) as nrm:
                zall16 = nrm.tile([4, S], f16, tag="z16", bufs=1)
                nc.vector.tensor_copy(zall16[:], zall_sb[:])
                for p in range(2):
                    zb = psz.tile([128, S], f32, tag="zb", name="zb")
                    for ch in range(0, S, 512):
                        nc.tensor.matmul(zb[:, ch:ch + 512],
                                         sel_sb[:, 128 * p:128 * p + 128],
                                         zall16[:, ch:ch + 512],
                                         start=True, stop=True)
                    lnz = nrm.tile([128, S], f32, tag="lnz", name="lnz")
                    nc.scalar.activation(lnz[:], zb[:], Act.Ln)
                    rz = nrm.tile([128, S], f32, tag="rz", name="rz")
                    nc.scalar.activation(rz[:], lnz[:], Act.Exp, scale=-1.0)
                    nc.vector.tensor_tensor(ctxu_sb[:, p, :], ctxu_sb[:, p, :],
                                            rz[:], Alu.mult)

            # ---- phase 3: output projection ----
            with tc.tile_pool(name="ps_o", bufs=4, space="PSUM") as pso, \
                 tc.tile_pool(name="ot", bufs=4) as otp:
                for st in range(NST):
                    ssl = slice(st * 128, (st + 1) * 128)
                    for n in range(2):
                        nsl = slice(n * 512, (n + 1) * 512)
                        po = pso.tile([128, 512], f32, tag="po", name="po")
                        nc.tensor.matmul(po[:], ctxu_sb[:, 0, ssl],
                                         wout_sb[:, 0, nsl], start=True, stop=False)
                        nc.tensor.matmul(po[:], ctxu_sb[:, 1, ssl],
                                         wout_sb[:, 1, nsl], start=False, stop=True)
                        ot = otp.tile([128, 512], f32, tag="ot", name="ot")
                        nc.any.tensor_copy(ot[:], po[:])
                        nc.sync.dma_start(out_d[ssl, nsl], ot[:])
            paircm.__exit__(None, None, None)
            rotcm.__exit__(None, None, None)

    nc.compile()
    return nc


def _host_inputs(x, W_qkv, W_out):
    """Build the 8 per-core input maps."""
    x = np.asarray(x, dtype=np.float32)
    W_qkv = np.asarray(W_qkv, dtype=np.float32)
    W_out = np.asarray(W_out, dtype=np.float32)

    pos = np.arange(S)
    freqs = 1.0 / 10000.0 ** (np.arange(0, HEAD_DIM, 2) / HEAD_DIM)
    ang = pos[:, None] * freqs[None, :]            # (S, 32)
    cs32 = np.cos(ang).T.astype(np.float32)        # (32, S)
    sn32 = np.sin(ang).T.astype(np.float32)
    cs = np.tile(cs32, (4, 1))                     # (128, S)
    sn = np.tile(sn32, (4, 1))
    tri = (np.arange(128)[:, None] <= np.arange(128)[None, :]).astype(np.float16)
    # selector for Z broadcast: sel[k, 128p+m] = 1 where k == 2p + m//64
    sel = np.zeros((4, 256), np.float16)
    for p in range(2):
        for m in range(128):
            sel[2 * p + m // 64, 128 * p + m] = 1.0

    in_maps = []
    for b in range(B):
        xT = np.ascontiguousarray(x[b].T.astype(np.float16))
        for g in range(NG):
            heads = np.arange(HG * g, HG * g + HG)
            qa = np.concatenate([0 * NUM_HEADS * HEAD_DIM + h * HEAD_DIM
                                 + np.arange(0, HEAD_DIM, 2) for h in heads])
            qb = qa + 1
            ka = qa + NUM_HEADS * HEAD_DIM
            kb = ka + 1
            wqk = np.ascontiguousarray(
                W_qkv[:, np.concatenate([qa, qb, ka, kb])].astype(np.float16))
            vcols = np.concatenate([2 * NUM_HEADS * HEAD_DIM + h * HEAD_DIM
                                    + np.arange(HEAD_DIM) for h in heads])
            wv = np.ascontiguousarray(W_qkv[:, vcols].astype(np.float16))
            wout = np.ascontiguousarray(
                W_out[HG * g * HEAD_DIM:HG * (g + 1) * HEAD_DIM].astype(np.float16))
            in_maps.append({"xT": xT, "wqk": wqk, "wv": wv, "wout": wout,
                            "cs": cs, "sn": sn, "tri": tri, "sel": sel})
    return in_maps


def get_program():
    if "nc" not in _CACHE:
        _CACHE["nc"] = _build_program()
    return _CACHE["nc"]


def run(x, W_qkv, W_out, trace=False, tmpdir=None):
    from concourse import bass_utils
    nc = get_program()
    in_maps = _host_inputs(x, W_qkv, W_out)
    res = bass_utils.run_bass_kernel_spmd(
        nc, in_maps, core_ids=list(range(N_CORES)), trace=trace, tmpdir=tmpdir)
    out = np.zeros((B, S, E), np.float32)
    for b in range(B):
        for g in range(NG):
            out[b] += res.results[b * NG + g]["out"]
    return out, res


def kernel(x, W_qkv, W_out):
    out, _ = run(x, W_qkv, W_out)
    return out


# revision 14
# speedup vs baseline: 2.7857x; 1.4375x over previous
"""Causal self-attention (RoPE) Trainium2 kernel, 8-way sharded.

Sharding: core = (batch b in 0..1) x (head group g in 0..3, 4 heads each).
Each core computes its batch's attention for its 4 heads plus the partial
output projection; the host sums the 4 partials per batch.

Layout strategy (per core):
- host passes xT = x[b].T (fp16) so the embed dim lands on SBUF partitions.
- W_qkv columns are permuted so q^T/k^T emerge from the projection matmul
  already transposed, with RoPE even/odd dim pairs de-interleaved into
  x1/x2 partition blocks (scores are invariant to a head-dim permutation).
- all matmul operands are fp16 (1 cycle/row on PE vs 4 for fp32); PSUM
  accumulation stays fp32. End-to-end error ~5e-4.
- scores are computed transposed (sT[j,i]); softmax needs no max pass
  (|scores| < ~4) and the denominator is obtained by appending a ones
  column to V (M=65 PV matmuls). Normalization happens at the end via a
  selector-matmul broadcast of Z and Ln/Exp reciprocal on ACT.
- causal masking: only j<=i column ranges are computed; the single
  diagonal 128x128 block per j-tile gets a triangle multiply.
"""
import sys

sys.path.insert(0, "/opt/trn_rl_repo")

import numpy as np

NUM_HEADS = 16
HEAD_DIM = 64
B, S, E = 2, 2048, 1024
HG = 4                      # heads per core
NG = NUM_HEADS // HG        # head groups
N_CORES = B * NG
F_QK = 2 * HG * HEAD_DIM    # 512 projected q+k rows per core
F_V = HG * HEAD_DIM         # 256 v cols per core
ESUB = E // 128             # 8 K-subtiles over embed dim
NCHUNK = 4                  # 512-col seq chunks (projection)
CHUNK = S // NCHUNK         # 512
NST = S // 128              # 16 seq tiles of 128
BLK = 512                   # attention i-block width
NBLK = S // BLK             # 2

_CACHE = {}


def _build_program():
    import concourse.bass as bass
    import concourse.mybir as mybir
    import concourse.tile as tile
    from concourse import bacc

    f32 = mybir.dt.float32
    f16 = mybir.dt.float16
    Alu = mybir.AluOpType
    Act = mybir.ActivationFunctionType

    nc = bacc.Bacc("TRN2", target_bir_lowering=False, debug=False,
                   num_devices=N_CORES)

    xT_d = nc.dram_tensor("xT", (E, S), f16, kind="ExternalInput").ap()
    wqk_d = nc.dram_tensor("wqk", (E, F_QK), f16, kind="ExternalInput").ap()
    wv_d = nc.dram_tensor("wv", (E, F_V), f16, kind="ExternalInput").ap()
    wout_d = nc.dram_tensor("wout", (F_V, E), f16, kind="ExternalInput").ap()
    cs_d = nc.dram_tensor("cs", (128, S), f32, kind="ExternalInput").ap()
    sn_d = nc.dram_tensor("sn", (128, S), f32, kind="ExternalInput").ap()
    tri_d = nc.dram_tensor("tri", (128, 128), f16, kind="ExternalInput").ap()
    sel_d = nc.dram_tensor("sel", (4, 256), f16, kind="ExternalInput").ap()
    out_d = nc.dram_tensor("out", (S, E), f32, kind="ExternalOutput").ap()

    with tile.TileContext(nc) as tc:
        with tc.tile_pool(name="wc", bufs=1) as wpool, \
             tc.tile_pool(name="vctx", bufs=1) as vpool:
            # ---- small whole-kernel constants ----
            tri_sb = wpool.tile([128, 128], f16)
            nc.sync.dma_start(tri_sb[:], tri_d[:])
            sel_sb = wpool.tile([4, 256], f16)
            nc.sync.dma_start(sel_sb[:], sel_d[:])

            # v tiles with per-head ones column: [128, st, 4*65]
            v_sb = vpool.tile([128, NST, HG * 65], f16)
            nc.gpsimd.memset(v_sb[:], 1.0)
            # unnormalized ctx^T (fp16) + Z rows
            ctxu_sb = vpool.tile([128, 2, S], f16)
            zall_sb = vpool.tile([4, S], f32)

            # rope outputs (x1/x2 aligned); pair-interleaved after relayout
            rotcm = tc.tile_pool(name="rot", bufs=1)
            rotpool = rotcm.__enter__()
            # pair-interleaved tiles: rows [h_even x1 | h_even x2 | h_odd x1 | h_odd x2]
            paircm = tc.tile_pool(name="pairs", bufs=1)
            pairpool = paircm.__enter__()
            qp = pairpool.tile([128, 2, S], f16)   # pair-interleaved q^T
            kp = pairpool.tile([128, 2, S], f16)
            wout_sb = pairpool.tile([128, 2, E], f16)
            nc.sync.dma_start(wout_sb[:], wout_d.rearrange("(o p) e -> p o e", p=128))
            qra = rotpool.tile([128, S], f16)
            qrb = rotpool.tile([128, S], f16)
            kra = rotpool.tile([128, S], f16)
            krb = rotpool.tile([128, S], f16)

            # ---- phase 1: projections ----
            with tc.tile_pool(name="x", bufs=1) as xpool, \
                 tc.tile_pool(name="rsc", bufs=3) as rsc:
                wqk_sb = xpool.tile([128, ESUB, F_QK], f16)
                nc.sync.dma_start(wqk_sb[:], wqk_d.rearrange("(o p) f -> p o f", p=128))
                wv_sb = xpool.tile([128, ESUB, F_V], f16)
                nc.sync.dma_start(wv_sb[:], wv_d.rearrange("(o p) f -> p o f", p=128))
                cs_sb = xpool.tile([128, S], f32)
                nc.sync.dma_start(cs_sb[:], cs_d[:])
                sn_sb = xpool.tile([128, S], f32)
                nc.sync.dma_start(sn_sb[:], sn_d[:])
                xT_sb = xpool.tile([128, ESUB, S], f16)
                for c in range(NCHUNK):
                    csl = slice(c * CHUNK, (c + 1) * CHUNK)
                    nc.sync.dma_start(
                        xT_sb[:, :, csl],
                        xT_d.rearrange("(o p) s -> p o s", p=128)[:, :, csl])


                # v projection per seq tile: v[s,f] = xT.T @ Wv
                with tc.tile_pool(name="ps_v", bufs=2, space="PSUM") as psv:
                    for st in range(NST):
                        ssl = slice(st * 128, (st + 1) * 128)
                        pv = psv.tile([128, F_V], f32, tag="v")
                        for e in range(ESUB):
                            nc.tensor.matmul(pv[:], xT_sb[:, e, ssl], wv_sb[:, e, :],
                                             start=(e == 0), stop=(e == ESUB - 1))
                        # scatter heads into 65-wide slots (col 64 etc stay 1.0)
                        nc.vector.tensor_copy(
                            v_sb[:, st, :].rearrange("p (h w) -> p h w", h=HG)[:, :, 0:64],
                            pv[:].rearrange("p (h w) -> p h w", h=HG))

                # q/k projection + rope, chunked over seq
                with tc.tile_pool(name="ps_qk", bufs=2, space="PSUM") as psqk:
                    for c in range(NCHUNK):
                        csl = slice(c * CHUNK, (c + 1) * CHUNK)
                        pqa = psqk.tile([128, CHUNK], f32, tag="qa")
                        pqb = psqk.tile([128, CHUNK], f32, tag="qb")
                        pka = psqk.tile([128, CHUNK], f32, tag="ka")
                        pkb = psqk.tile([128, CHUNK], f32, tag="kb")
                        for e in range(ESUB):
                            kw = dict(start=(e == 0), stop=(e == ESUB - 1))
                            xs = xT_sb[:, e, csl]
                            nc.tensor.matmul(pqa[:], wqk_sb[:, e, 0:128], xs, **kw)
                            nc.tensor.matmul(pqb[:], wqk_sb[:, e, 128:256], xs, **kw)
                            nc.tensor.matmul(pka[:], wqk_sb[:, e, 256:384], xs, **kw)
                            nc.tensor.matmul(pkb[:], wqk_sb[:, e, 384:512], xs, **kw)
                        for (pa, pb, ra, rb) in ((pqa, pqb, qra, qrb),
                                                 (pka, pkb, kra, krb)):
                            t1 = rsc.tile([128, CHUNK], f32, tag="t1", name="t1")
                            t2 = rsc.tile([128, CHUNK], f32, tag="t2", name="t2")
                            nc.vector.tensor_tensor(t1[:], pa[:], cs_sb[:, csl], Alu.mult)
                            nc.vector.tensor_tensor(t2[:], pb[:], sn_sb[:, csl], Alu.mult)
                            nc.vector.tensor_tensor(ra[:, csl], t1[:], t2[:], Alu.subtract)
                            t3 = rsc.tile([128, CHUNK], f32, tag="t1", name="t3")
                            t4 = rsc.tile([128, CHUNK], f32, tag="t2", name="t4")
                            nc.vector.tensor_tensor(t3[:], pa[:], sn_sb[:, csl], Alu.mult)
                            nc.vector.tensor_tensor(t4[:], pb[:], cs_sb[:, csl], Alu.mult)
                            nc.vector.tensor_tensor(rb[:, csl], t3[:], t4[:], Alu.add)
                        for p in range(2):
                            h0, h1 = 2 * p, 2 * p + 1
                            for (ra, rb, dst) in ((qra, qrb, qp), (kra, krb, kp)):
                                nc.sync.dma_start(dst[0:32, p, csl],
                                                  ra[32 * h0:32 * h0 + 32, csl])
                                nc.sync.dma_start(dst[32:64, p, csl],
                                                  rb[32 * h0:32 * h0 + 32, csl])
                                nc.sync.dma_start(dst[64:96, p, csl],
                                                  ra[32 * h1:32 * h1 + 32, csl])
                                nc.sync.dma_start(dst[96:128, p, csl],
                                                  rb[32 * h1:32 * h1 + 32, csl])

            # ---- phase 2: attention ----
            scale = 1.0 / float(np.sqrt(HEAD_DIM))
            with tc.tile_pool(name="ps_at", bufs=1, space="PSUM") as psat, \
                 tc.tile_pool(name="pt", bufs=2) as ptp:
                for bb in range(NBLK):
                    i0 = bb * BLK
                    njt = 4 * bb + 4
                    for p in range(2):
                        ctx = [psat.tile([65, BLK], f32, tag=f"ctx{a}",
                                         name=f"ctx{a}", bufs=2)
                               for a in range(2)]
                        for jt in range(njt):
                            r = jt - 4 * bb
                            off = 128 * max(r, 0)
                            w = BLK - off
                            # both heads' scores in one [128, 2, BLK] psum
                            ps_s = psat.tile([128, 2, BLK], f32, tag="s",
                                             name="ps_s", bufs=2)
                            chunks = []
                            c0 = off
                            while c0 < BLK:
                                c1 = min(BLK, (c0 // 512 + 1) * 512)
                                chunks.append((c0, c1 - c0))
                                c0 = c1
                            for a in range(2):
                                for ch, cw in chunks:
                                    nc.tensor.matmul(
                                        ps_s[:, a, ch:ch + cw],
                                        kp[64 * a:64 * a + 64, p,
                                           128 * jt:128 * jt + 128],
                                        qp[64 * a:64 * a + 64, p,
                                           i0 + ch:i0 + ch + cw],
                                        start=True, stop=True)
                            pt = ptp.tile([128, 2, BLK], f16, tag="pt", name="pt")
                            nc.scalar.activation(pt[:, :, off:], ps_s[:, :, off:],
                                                 Act.Exp, scale=scale)
                            if r >= 0:
                                nc.vector.tensor_tensor(
                                    pt[:, :, off:off + 128],
                                    pt[:, :, off:off + 128],
                                    tri_sb[:, None, :].to_broadcast((128, 2, 128)),
                                    Alu.mult)
                            # per-element has_written handles the ragged
                            # causal column ranges; the 2KB-granularity group
                            # check cannot express them, so skip it
                            for a in range(2):
                                for ch, cw in chunks:
                                    nc.tensor.matmul(
                                        ctx[a][:, ch:ch + cw],
                                        v_sb[:, jt, 65 * (2 * p + a):
                                             65 * (2 * p + a) + 65],
                                        pt[:, a, ch:ch + cw],
                                        start=(jt == 0), stop=(jt == njt - 1),
                                        skip_group_check=True)
                        # stash unnormalized ctx + Z rows
                        for a in range(2):
                            nc.vector.tensor_copy(
                                ctxu_sb[64 * a:64 * a + 64, p, i0:i0 + BLK],
                                ctx[a][0:64, :])
                            zrow = ptp.tile([1, BLK], f32, tag="zrow",
                                            name="zrow")
                            nc.vector.tensor_copy(zrow[:], ctx[a][64:65, :])
                            nc.sync.dma_start(
                                zall_sb[2 * p + a:2 * p + a + 1, i0:i0 + BLK],
                                zrow[:])

            # ---- phase 2.5: batched normalization ----
            with tc.tile_pool(name="ps_z", bufs=2, space="PSUM") as psz, \
                 tc.tile_pool(name="nrm", bufs=2) as nrm:
                zall16 = nrm.tile([4, S], f16, tag="z16", bufs=1)
                nc.vector.tensor_copy(zall16[:], zall_sb[:])
                for p in range(2):
                    zb = psz.tile([128, S], f32, tag="zb", name="zb")
                    for ch in range(0, S, 512):
                        nc.tensor.matmul(zb[:, ch:ch + 512],
                                         sel_sb[:, 128 * p:128 * p + 128],
                                         zall16[:, ch:ch + 512],
                                         start=True, stop=True)
                    lnz = nrm.tile([128, S], f32, tag="lnz", name="lnz")
                    nc.scalar.activation(lnz[:], zb[:], Act.Ln)
                    rz = nrm.tile([128, S], f32, tag="rz", name="rz")
                    nc.scalar.activation(rz[:], lnz[:], Act.Exp, scale=-1.0)
                    nc.vector.tensor_tensor(ctxu_sb[:, p, :], ctxu_sb[:, p, :],
                                            rz[:], Alu.mult)

            # ---- phase 3: output projection ----
            with tc.tile_pool(name="ps_o", bufs=4, space="PSUM") as pso, \
                 tc.tile_pool(name="ot", bufs=4) as otp:
                for st in range(NST):
                    ssl = slice(st * 128, (st + 1) * 128)
                    for n in range(2):
                        nsl = slice(n * 512, (n + 1) * 512)
                        po = pso.tile([128, 512], f32, tag="po", name="po")
                        nc.tensor.matmul(po[:], ctxu_sb[:, 0, ssl],
                                         wout_sb[:, 0, nsl], start=True, stop=False)
                        nc.tensor.matmul(po[:], ctxu_sb[:, 1, ssl],
                                         wout_sb[:, 1, nsl], start=False, stop=True)
                        ot = otp.tile([128, 512], f32, tag="ot", name="ot")
                        nc.any.tensor_copy(ot[:], po[:])
                        nc.sync.dma_start(out_d[ssl, nsl], ot[:])
            paircm.__exit__(None, None, None)
            rotcm.__exit__(None, None, None)

    nc.compile()
    return nc


def _host_inputs(x, W_qkv, W_out):
    """Build the 8 per-core input maps."""
    x = np.asarray(x, dtype=np.float32)
    W_qkv = np.asarray(W_qkv, dtype=np.float32)
    W_out = np.asarray(W_out, dtype=np.float32)

    pos = np.arange(S)
    freqs = 1.0 / 10000.0 ** (np.arange(0, HEAD_DIM, 2) / HEAD_DIM)
    ang = pos[:, None] * freqs[None, :]            # (S, 32)
    cs32 = np.cos(ang).T.astype(np.float32)        # (32, S)
    sn32 = np.sin(ang).T.astype(np.float32)
    cs = np.tile(cs32, (4, 1))                     # (128, S)
    sn = np.tile(sn32, (4, 1))
    tri = (np.arange(128)[:, None] <= np.arange(128)[None, :]).astype(np.float16)
    # selector for Z broadcast: sel[k, 128p+m] = 1 where k == 2p + m//64
    sel = np.zeros((4, 256), np.float16)
    for p in range(2):
        for m in range(128):
            sel[2 * p + m // 64, 128 * p + m] = 1.0

    in_maps = []
    for b in range(B):
        xT = np.ascontiguousarray(x[b].T.astype(np.float16))
        for g in range(NG):
            heads = np.arange(HG * g, HG * g + HG)
            qa = np.concatenate([0 * NUM_HEADS * HEAD_DIM + h * HEAD_DIM
                                 + np.arange(0, HEAD_DIM, 2) for h in heads])
            qb = qa + 1
            ka = qa + NUM_HEADS * HEAD_DIM
            kb = ka + 1
            wqk = np.ascontiguousarray(
                W_qkv[:, np.concatenate([qa, qb, ka, kb])].astype(np.float16))
            vcols = np.concatenate([2 * NUM_HEADS * HEAD_DIM + h * HEAD_DIM
                                    + np.arange(HEAD_DIM) for h in heads])
            wv = np.ascontiguousarray(W_qkv[:, vcols].astype(np.float16))
            wout = np.ascontiguousarray(
                W_out[HG * g * HEAD_DIM:HG * (g + 1) * HEAD_DIM].astype(np.float16))
            in_maps.append({"xT": xT, "wqk": wqk, "wv": wv, "wout": wout,
                            "cs": cs, "sn": sn, "tri": tri, "sel": sel})
    return in_maps


def get_program():
    if "nc" not in _CACHE:
        _CACHE["nc"] = _build_program()
    return _CACHE["nc"]


def run(x, W_qkv, W_out, trace=False, tmpdir=None):
    from concourse import bass_utils
    nc = get_program()
    in_maps = _host_inputs(x, W_qkv, W_out)
    res = bass_utils.run_bass_kernel_spmd(
        nc, in_maps, core_ids=list(range(N_CORES)), trace=trace, tmpdir=tmpdir)
    out = np.zeros((B, S, E), np.float32)
    for b in range(B):
        for g in range(NG):
            out[b] += res.results[b * NG + g]["out"]
    return out, res


def kernel(x, W_qkv, W_out):
    out, _ = run(x, W_qkv, W_out)
    return out
